# revision 5
# baseline (speedup 1.0000x reference)
"""Trainium2 Bass kernel for nn_AttentionBlock (B=8, S=2048, D=1024).

Reference computation (per batch element b):
    q = x @ Wq + bq ; k = x @ Wk + bk ; v = x @ Wv + bv
    scores = (q @ k^T) / sqrt(1024)
    attn = softmax(scores, axis=QUERY)          # axis=1 of [B, S_q, S_k]!
    out = attn @ v

Sharding: pure data-parallel — batch element b runs on NeuronCore b.

Device algorithm (all matmuls bf16 inputs, fp32 PSUM accumulation):
  - host supplies x^T (bf16, tile layout) so every projection is a plain
    `out = lhsT.T @ rhs` with the contraction (emb) on partitions.
  - scores are computed TRANSPOSED: sT[j, i] = k_j . q_i, so the softmax
    reduction axis (i = query) is the free axis.  The scaled scores lie in
    [-2.2, 2.2] for this data distribution, so softmax needs no max
    subtraction: E = exp(s * scale), Z = sum_i E — both produced by a single
    ScalarE activation pass (accum_out).  1/Z is folded into v rows:
    out[i, :] = sum_j E^T[j, i] * (v[j, :] / Z_j).
"""

import numpy as np
import ml_dtypes

S = 2048          # sequence length
E = 1024          # emb dim == att dim
P = 128           # partitions
NS = S // P       # 16 sequence tiles
NE = E // P       # 8 emb tiles
NCORES = 8
SCALE = 1.0 / 32.0  # 1/sqrt(1024)

_BUILT = {}


def _build():
    """Construct the Bass program (same NEFF for all 8 cores)."""
    from contextlib import ExitStack
    import concourse.tile as tile
    import concourse.mybir as mybir
    from concourse import bacc

    f32 = mybir.dt.float32
    bf16 = mybir.dt.bfloat16
    Act = mybir.ActivationFunctionType

    nc = bacc.Bacc("TRN2", target_bir_lowering=False, debug=False)

    xT_d = nc.dram_tensor("xT", [P, NE, S], bf16, kind="ExternalInput").ap()
    wq_d = nc.dram_tensor("Wq", [P, NE, E], bf16, kind="ExternalInput").ap()
    wk_d = nc.dram_tensor("Wk", [P, NE, E], bf16, kind="ExternalInput").ap()
    wv_d = nc.dram_tensor("Wv", [P, NE, E], bf16, kind="ExternalInput").ap()
    bqk_d = nc.dram_tensor("bqk", [P, 2 * NE], f32, kind="ExternalInput").ap()
    bv_d = nc.dram_tensor("bv", [1, E], bf16, kind="ExternalInput").ap()
    out_d = nc.dram_tensor("out", [S, E], f32, kind="ExternalOutput").ap()

    with tile.TileContext(nc) as tc, ExitStack() as ctx:
        const_p = ctx.enter_context(tc.tile_pool(name="const", bufs=1))
        bqk_t = const_p.tile([P, 2 * NE], f32)
        bv_t = const_p.tile([1, E], bf16)
        ones_t = const_p.tile([1, P], bf16)
        zz = const_p.tile([P, NS], f32)
        zr = const_p.tile([P, NS], f32)

        qT_p = ctx.enter_context(tc.tile_pool(name="qT", bufs=1))
        qT = qT_p.tile([P, NE, S], bf16)
        kT_p = ctx.enter_context(tc.tile_pool(name="kT", bufs=1))
        kT = kT_p.tile([P, NE, S], bf16)
        v_p = ctx.enter_context(tc.tile_pool(name="v", bufs=1))
        v_t = v_p.tile([P, NS, E], bf16)

        ps = ctx.enter_context(tc.tile_pool(name="ps", bufs=2, space="PSUM"))

        with ExitStack() as ph1:
            xT_p = ph1.enter_context(tc.tile_pool(name="xT", bufs=1))
            xT = xT_p.tile([P, NE, S], bf16)
            for e in range(NE):
                nc.sync.dma_start(xT[:, e, :], xT_d[:, e, :])
            w_p = ph1.enter_context(tc.tile_pool(name="w", bufs=3))

            # ---- v = x @ Wv + bv : v_t[:, j, :] = v[j*P:(j+1)*P, :] ----
            wv_t = w_p.tile([P, NE, E], bf16, tag="w")
            wq_t = w_p.tile([P, NE, E], bf16, tag="w")
            wk_t = w_p.tile([P, NE, E], bf16, tag="w")
            for e in range(NE):
                nc.sync.dma_start(wv_t[:, e, :], wv_d[:, e, :])
            nc.sync.dma_start(bqk_t[:], bqk_d)
            nc.sync.dma_start(bv_t[:], bv_d)
            nc.vector.memset(ones_t[:], 1.0)
            for e in range(NE):
                nc.sync.dma_start(wq_t[:, e, :], wq_d[:, e, :])
            for e in range(NE):
                nc.sync.dma_start(wk_t[:, e, :], wk_d[:, e, :])
            for j in range(NS):
                pv = ps.tile([P, S], f32, tag="ps")
                for e in range(NE):
                    lhsT = xT[:, e, j * P:(j + 1) * P]
                    for c in range(2):
                        cs = slice(c * 512, (c + 1) * 512)
                        nc.tensor.matmul(pv[:, cs], lhsT, wv_t[:, e, cs],
                                         start=(e == 0), stop=False)
                # bias via rank-1 matmul: ones[1,P].T @ bv[1,E]
                for c in range(2):
                    cs = slice(c * 512, (c + 1) * 512)
                    nc.tensor.matmul(pv[:, cs], ones_t[0:1, 0:P], bv_t[0:1, cs],
                                     start=False, stop=True)
                nc.vector.tensor_copy(v_t[:, j, :], pv[:, 0:E])

            # ---- qT / kT : outT[:, d, :] = (x @ W + b).T  d-tile rows ----
            for (w_t, bias_off, outT) in ((wq_t, 0, qT), (wk_t, NE, kT)):
                for d in range(NE):
                    pq = ps.tile([P, S], f32, tag="ps")
                    for e in range(NE):
                        lhsT = w_t[:, e, d * P:(d + 1) * P]
                        for c in range(4):
                            cs = slice(c * 512, (c + 1) * 512)
                            nc.tensor.matmul(pq[:, cs], lhsT, xT[:, e, cs],
                                             start=(e == 0), stop=(e == NE - 1))
                    bcol = bqk_t[:, bias_off + d:bias_off + d + 1]
                    nc.scalar.activation(outT[:, d, :], pq[:, :],
                                         func=Act.Identity, bias=bcol, scale=1.0)

        # ---- scoresT + softmax-over-query + fold 1/Z into v ----
        Et_p = ctx.enter_context(tc.tile_pool(name="Et", bufs=1))
        Et = Et_p.tile([P, NS, S], bf16)
        for j in range(NS):
            pss = ps.tile([P, S], f32, tag="ps")
            for d in range(NE):
                lhsT = kT[:, d, j * P:(j + 1) * P]
                for c in range(4):
                    cs = slice(c * 512, (c + 1) * 512)
                    nc.tensor.matmul(pss[:, cs], lhsT, qT[:, d, cs],
                                     start=(d == 0), stop=(d == NE - 1))
            nc.scalar.activation(Et[:, j, :], pss[:, :], func=Act.Exp,
                                 scale=SCALE, accum_out=zz[:, j:j + 1])
            nc.vector.reciprocal(zr[:, j:j + 1], zz[:, j:j + 1])
            nc.vector.tensor_scalar_mul(v_t[:, j, :], v_t[:, j, :],
                                        zr[:, j:j + 1])

        # ---- out[i, :] = sum_j E^T[j, i-tile] . v'[j] ----
        ost_p = ctx.enter_context(tc.tile_pool(name="ost", bufs=3))
        for i in range(NS):
            po = ps.tile([P, S], f32, tag="ps")
            for j in range(NS):
                lhsT = Et[:, j, i * P:(i + 1) * P]
                for c in range(2):
                    cs = slice(c * 512, (c + 1) * 512)
                    nc.tensor.matmul(po[:, cs], lhsT, v_t[:, j, cs],
                                     start=(j == 0), stop=(j == NS - 1))
            ob = ost_p.tile([P, E], f32, tag="ost")
            nc.vector.tensor_copy(ob[:], po[:, 0:E])
            nc.sync.dma_start(out_d[i * P:(i + 1) * P, :], ob[:])

    nc.compile()
    return nc


def _get_built():
    if "nc" not in _BUILT:
        _BUILT["nc"] = _build()
    return _BUILT["nc"]


def _prep_wT(w):
    # [E, E] (in, out) -> tile layout [P, NE, E]: [p, e, d] = W[e*P + p, d]
    return np.ascontiguousarray(
        w.reshape(NE, P, E).transpose(1, 0, 2)).astype(ml_dtypes.bfloat16)


def kernel(**inputs):
    x = np.asarray(inputs["x_h"], dtype=np.float32)     # [8, S, E]
    Wq = np.asarray(inputs["Wq"], dtype=np.float32)
    bq = np.asarray(inputs["bq"], dtype=np.float32)
    Wk = np.asarray(inputs["Wk"], dtype=np.float32)
    bk = np.asarray(inputs["bk"], dtype=np.float32)
    Wv = np.asarray(inputs["Wv"], dtype=np.float32)
    bv = np.asarray(inputs["bv"], dtype=np.float32)

    from concourse.bass_utils import run_bass_kernel_spmd

    nc = _get_built()

    wq_h = _prep_wT(Wq)
    wk_h = _prep_wT(Wk)
    wv_h = _prep_wT(Wv)
    # bqk[p, d] = bq[d*P + p]; bqk[p, NE + d] = bk[d*P + p]
    bqk_h = np.ascontiguousarray(
        np.concatenate([bq.reshape(NE, P).T, bk.reshape(NE, P).T], axis=1)
    ).astype(np.float32)
    bv_h = bv.reshape(1, E).astype(ml_dtypes.bfloat16)

    in_maps = []
    for b in range(NCORES):
        # xT tile layout [P, NE, S]: [p, e, i] = x[b][i, e*P + p]
        xT_h = np.ascontiguousarray(
            x[b].T.reshape(NE, P, S).transpose(1, 0, 2)
        ).astype(ml_dtypes.bfloat16)
        in_maps.append({
            "xT": xT_h, "Wq": wq_h, "Wk": wk_h, "Wv": wv_h,
            "bqk": bqk_h, "bv": bv_h,
        })

    res = run_bass_kernel_spmd(nc, in_maps, list(range(NCORES)))
    out = np.stack([np.asarray(res.results[b]["out"], dtype=np.float32)
                    for b in range(NCORES)])
    return out


# revision 12
# speedup vs baseline: 62.7974x; 62.7974x over previous
"""Trainium2 Bass kernel for nn_AttentionBlock (B=8, S=2048, D=1024).

Reference computation (per batch element b):
    q = x @ Wq + bq ; k = x @ Wk + bk ; v = x @ Wv + bv
    scores = (q @ k^T) / sqrt(1024)
    attn = softmax(scores, axis=QUERY)          # axis=1 of [B, S_q, S_k]!
    out = attn @ v

Sharding: pure data-parallel — batch element b runs on NeuronCore b.

Device algorithm (all matmuls bf16 inputs, fp32 PSUM accumulation):
  - host supplies x^T (bf16, tile layout) so every projection is a plain
    `out = lhsT.T @ rhs` with the contraction (emb) on partitions.
  - scores are computed TRANSPOSED: sT[j, i] = k_j . q_i, so the softmax
    reduction axis (i = query) is the free axis.  The scaled scores lie in
    [-2.2, 2.2] for this data distribution, so softmax needs no max
    subtraction: E = exp(s * scale), Z = sum_i E — both produced by a single
    ScalarE activation pass (accum_out).  1/Z is folded into v rows:
    out[i, :] = sum_j E^T[j, i] * (v[j, :] / Z_j).
"""

import numpy as np
import ml_dtypes

S = 2048          # sequence length
E = 1024          # emb dim == att dim
P = 128           # partitions
NS = S // P       # 16 sequence tiles
NE = E // P       # 8 emb tiles
NCORES = 8
SCALE = 1.0 / 32.0  # 1/sqrt(1024)

_BUILT = {}


def _build(reps=1):
    """Construct the Bass program (same NEFF for all 8 cores).

    reps>1 emits the body multiple times back-to-back (benchmarking only:
    wall(K) - wall(1) = (K-1) * body time, cancelling launch/transfer
    overhead that dominates wall measurements through the axon tunnel).
    """
    from contextlib import ExitStack
    import concourse.tile as tile
    import concourse.mybir as mybir
    from concourse import bacc

    nc = bacc.Bacc("TRN2", target_bir_lowering=False, debug=False)

    f32 = mybir.dt.float32
    bf16 = mybir.dt.bfloat16

    xT_d = nc.dram_tensor("xT", [P, NE, S], bf16, kind="ExternalInput").ap()
    wq_d = nc.dram_tensor("Wq", [P, NE, E], bf16, kind="ExternalInput").ap()
    wk_d = nc.dram_tensor("Wk", [P, NE, E], bf16, kind="ExternalInput").ap()
    wv_d = nc.dram_tensor("Wv", [P, NE, E], bf16, kind="ExternalInput").ap()
    bqk_d = nc.dram_tensor("bqk", [P, 2 * NE], f32, kind="ExternalInput").ap()
    bv_d = nc.dram_tensor("bv", [P, E], bf16, kind="ExternalInput").ap()
    out_d = nc.dram_tensor("out", [S, E], f32, kind="ExternalOutput").ap()

    with tile.TileContext(nc) as tc:
        for _ in range(reps):
            _emit_body(nc, tc, xT_d, wq_d, wk_d, wv_d, bqk_d, bv_d, out_d)

    nc.compile()
    return nc


def _emit_body(nc, tc, xT_d, wq_d, wk_d, wv_d, bqk_d, bv_d, out_d):
    from contextlib import ExitStack
    import concourse.mybir as mybir

    f32 = mybir.dt.float32
    bf16 = mybir.dt.bfloat16
    Act = mybir.ActivationFunctionType

    with ExitStack() as ctx:
        const_p = ctx.enter_context(tc.tile_pool(name="const", bufs=1))
        bqk_t = const_p.tile([P, 2 * NE], f32)
        bv_t = const_p.tile([P, E], bf16)
        zz = const_p.tile([P, NS], f32)
        zr = const_p.tile([P, NS], f32)

        qT_p = ctx.enter_context(tc.tile_pool(name="qT", bufs=1))
        qT = qT_p.tile([P, NE, S], bf16)
        kT_p = ctx.enter_context(tc.tile_pool(name="kT", bufs=1))
        kT = kT_p.tile([P, NE, S], bf16)
        v_p = ctx.enter_context(tc.tile_pool(name="v", bufs=1))
        v_t = v_p.tile([P, NS, E], bf16)

        ps = ctx.enter_context(tc.tile_pool(name="ps", bufs=2, space="PSUM"))

        with ExitStack() as ph1:
            nc.sync.dma_start(bqk_t[:], bqk_d)
            nc.sync.dma_start(bv_t[:], bv_d)
            xT_p = ph1.enter_context(tc.tile_pool(name="xT", bufs=NE))
            xts = []
            for e in range(NE):
                t = xT_p.tile([P, S], bf16, tag="xt")
                nc.sync.dma_start(t[:], xT_d[:, e, :])
                xts.append(t)
            w_p = ph1.enter_context(tc.tile_pool(name="w", bufs=3 * NE))

            # ---- v = x @ Wv + bv : v_t[:, j, :] = v[j*P:(j+1)*P, :] ----
            wvs, wqs, wks = [], [], []
            for (wl, wd) in ((wvs, wv_d), (wqs, wq_d), (wks, wk_d)):
                for e in range(NE):
                    t = w_p.tile([P, E], bf16, tag="w")
                    nc.sync.dma_start(t[:], wd[:, e, :])
                    wl.append(t)
            for j in range(NS):
                pv = ps.tile([P, S], f32, tag="ps")
                for e in range(NE):
                    lhsT = xts[e][:, j * P:(j + 1) * P]
                    for c in range(2):
                        cs = slice(c * 512, (c + 1) * 512)
                        nc.tensor.matmul(pv[:, cs], lhsT, wvs[e][:, cs],
                                         start=(e == 0), stop=(e == NE - 1))
                # fused bias add + cast during PSUM -> SBUF
                nc.vector.tensor_tensor(v_t[:, j, :], pv[:, 0:E], bv_t[:],
                                        op=mybir.AluOpType.add)

            # ---- qT / kT : outT[:, d, :] = (x @ W + b).T  d-tile rows ----
            for (ws, bias_off, outT) in ((wqs, 0, qT), (wks, NE, kT)):
                for d in range(NE):
                    pq = ps.tile([P, S], f32, tag="ps")
                    for e in range(NE):
                        lhsT = ws[e][:, d * P:(d + 1) * P]
                        for c in range(4):
                            cs = slice(c * 512, (c + 1) * 512)
                            nc.tensor.matmul(pq[:, cs], lhsT, xts[e][:, cs],
                                             start=(e == 0), stop=(e == NE - 1))
                    bcol = bqk_t[:, bias_off + d:bias_off + d + 1]
                    nc.scalar.activation(outT[:, d, :], pq[:, :],
                                         func=Act.Identity, bias=bcol, scale=1.0)

        # ---- scoresT + softmax-over-query + fold 1/Z into v ----
        Et_p = ctx.enter_context(tc.tile_pool(name="Et", bufs=1))
        Et = Et_p.tile([P, NS, S], bf16)
        for j in range(NS):
            pss = ps.tile([P, S], f32, tag="ps")
            for d in range(NE):
                lhsT = kT[:, d, j * P:(j + 1) * P]
                for c in range(4):
                    cs = slice(c * 512, (c + 1) * 512)
                    nc.tensor.matmul(pss[:, cs], lhsT, qT[:, d, cs],
                                     start=(d == 0), stop=(d == NE - 1))
            nc.scalar.activation(Et[:, j, :], pss[:, :], func=Act.Exp,
                                 scale=SCALE, accum_out=zz[:, j:j + 1])
            nc.vector.reciprocal(zr[:, j:j + 1], zz[:, j:j + 1])
            nc.vector.tensor_scalar_mul(v_t[:, j, :], v_t[:, j, :],
                                        zr[:, j:j + 1])

        # ---- out[i, :] = sum_j E^T[j, i-tile] . v'[j] ----
        ost_p = ctx.enter_context(tc.tile_pool(name="ost", bufs=3))
        for i in range(NS):
            po = ps.tile([P, S], f32, tag="ps")
            for j in range(NS):
                lhsT = Et[:, j, i * P:(i + 1) * P]
                for c in range(2):
                    cs = slice(c * 512, (c + 1) * 512)
                    nc.tensor.matmul(po[:, cs], lhsT, v_t[:, j, cs],
                                     start=(j == 0), stop=(j == NS - 1))
            ob = ost_p.tile([P, E], f32, tag="ost")
            nc.vector.tensor_copy(ob[:], po[:, 0:E])
            nc.sync.dma_start(out_d[i * P:(i + 1) * P, :], ob[:])


def _get_built():
    if "nc" not in _BUILT:
        _BUILT["nc"] = _build()
    return _BUILT["nc"]


def _prep_wT(w):
    # [E, E] (in, out) -> tile layout [P, NE, E]: [p, e, d] = W[e*P + p, d]
    return np.ascontiguousarray(
        w.reshape(NE, P, E).transpose(1, 0, 2)).astype(ml_dtypes.bfloat16)


def kernel(**inputs):
    x = np.asarray(inputs["x_h"], dtype=np.float32)     # [8, S, E]
    Wq = np.asarray(inputs["Wq"], dtype=np.float32)
    bq = np.asarray(inputs["bq"], dtype=np.float32)
    Wk = np.asarray(inputs["Wk"], dtype=np.float32)
    bk = np.asarray(inputs["bk"], dtype=np.float32)
    Wv = np.asarray(inputs["Wv"], dtype=np.float32)
    bv = np.asarray(inputs["bv"], dtype=np.float32)

    from concourse.bass_utils import run_bass_kernel_spmd

    nc = _get_built()

    wq_h = _prep_wT(Wq)
    wk_h = _prep_wT(Wk)
    wv_h = _prep_wT(Wv)
    # bqk[p, d] = bq[d*P + p]; bqk[p, NE + d] = bk[d*P + p]
    bqk_h = np.ascontiguousarray(
        np.concatenate([bq.reshape(NE, P).T, bk.reshape(NE, P).T], axis=1)
    ).astype(np.float32)
    bv_h = np.ascontiguousarray(
        np.broadcast_to(bv.reshape(1, E), (P, E))).astype(ml_dtypes.bfloat16)

    in_maps = []
    for b in range(NCORES):
        # xT tile layout [P, NE, S]: [p, e, i] = x[b][i, e*P + p]
        xT_h = np.ascontiguousarray(
            x[b].T.reshape(NE, P, S).transpose(1, 0, 2)
        ).astype(ml_dtypes.bfloat16)
        in_maps.append({
            "xT": xT_h, "Wq": wq_h, "Wk": wk_h, "Wv": wv_h,
            "bqk": bqk_h, "bv": bv_h,
        })

    res = run_bass_kernel_spmd(nc, in_maps, list(range(NCORES)))
    out = np.stack([np.asarray(res.results[b]["out"], dtype=np.float32)
                    for b in range(NCORES)])
    return out
